# revision 48
# baseline (speedup 1.0000x reference)
"""Trainium2 Bass kernel for nn_MultiHeadAttention_62783831933512.

Reference semantics (all H=12 heads share ONE set of projection weights,
so the module degenerates to single-head attention):
    qh = q @ Wq + bq ; kh = k @ Wk + bk ; vh = v @ Wv + bv          [B,S,64]
    scores  = (qh @ kh^T) / sqrt(H)                                  [B,S,S]
    weights = softmax(scores)
    oh      = weights @ vh
    out     = tile(oh, H) @ Wc + bc  ==  oh @ Wc_sum + bc
    attn    = broadcast(weights) -> [B,H,S,S]

Device-side structure (exact up to fp rounding):
  * Wc_sum = sum of the H row-blocks of Wc (host-precomputed, [64,768]) -
    the concat of H identical heads collapses the output projection 12x.
  * attn is H identical copies -> device computes the [B,S,S] weights
    once; the host returns a broadcast VIEW for the [B,H,S,S] output.
  * bk drops out of softmax (adds a per-query-row constant); bv and bc
    are folded in on the host after the projection (softmax rows sum to
    1); bq is added on-device to qh^T as a per-partition bias.
  * scores are built TRANSPOSED ([k,q] layout, k on partitions) so the
    exp tiles feed the AV matmul directly with no on-chip transpose of
    the big matrix; the softmax denominator comes free by augmenting vh
    with a ones column ([k,65] @ [k,q] -> row 64 = exp-sums).
  * no row-max subtraction: exp(s*x - C) with constant C; the shift
    cancels in the host-side normalization (w = wT_u / sums), and the
    exp arguments are bounded (|s*x| < 16) so fp32/fp16 ranges are safe.
  * the host transposes q/k/v slabs to [768, S] (contraction dim must be
    the SBUF partition dim; there is no fp32 DMA transpose on trn2).

Sharding over the 8 cores (SPMD, one NEFF): core c -> (batch c//2,
query half c%2).  Each core reads qT [768,512], kT/vT [768,1024] (k/v
shared by the core pair), writes out_u [512,768] (unnormalized),
wT_u [1024,512] (exp values), sums [1,512].  Host divides by sums.

Dtype modes (MHA_MODE env, default "f16"):
  f16  - q/k/v/W inputs quantized to fp16 on host (halves input DMA,
         full-rate matmuls with exact fp32 accumulation); qhT/khT/ohT/
         Wcs intermediates in float32r; exp tiles + outputs fp16.
         Measured end-to-end absmax-relative error vs the fp32
         reference: ~1.2e-3.
  f32r - fp32 storage everywhere, relaxed-precision (float32r) full-rate
         matmuls: ~7e-4 error, ~1.6x slower (more DMA bytes).
  f32  - exact fp32 matmuls (4 cyc/row): ~3e-6 error, slowest.

Schedule notes: inputs stream in dependency-chain order (Wqkv
host-pre-swizzled for contiguous DMA, then qT, kT halves with the first
vT quarter interleaved, remaining vT quarters, Wcs last); a few
throwaway fp32 matmuls warm the PE clock-gate and a dummy exp preloads
the ACT LUT during the stream; the score/exp chain overlaps the v-path
projection+transpose; all 8 PSUM banks are budgeted explicitly (4
shared acc/score slots + 2 v slots + 2 transpose slots, each quarter's
two transposes sharing one bank); outputs stream back in pairs on the
idle sync HWDGE ring.
"""

import math
from contextlib import ExitStack

import numpy as np

import concourse.bass as bass
import concourse.tile as tile
from concourse import bacc
from concourse import mybir
from concourse.masks import make_identity

B, S, D, H, DEPTH = 4, 1024, 768, 12, 64
NCORES = 8
QB = S // 2          # 512 query rows per core
NQT = QB // 128      # 4 query tiles
NKT = S // 128       # 8 key tiles
NDT = D // 128       # 6 contraction tiles over d_model
SCALE = 1.0 / math.sqrt(H)
ESHIFT = 8.0         # exp-arg shift; cancels in normalization

F32 = mybir.dt.float32
F32R = mybir.dt.float32r

F16 = mybir.dt.float16

# Matmul operand dtype mode:
#   "f16"  - inputs/weights quantized to fp16 on host; all matmuls fp16
#            (full rate, exact fp32 accumulation of fp16 products);
#            wT_u output stored fp16.  Halves input DMA bytes.
#   "f32r" - fp32 storage, relaxed-precision full-rate matmuls
#   "f32"  - exact fp32 matmuls (4 cyc/row)
import os as _os
MODE = _os.environ.get("MHA_MODE", "f16")


def build_bass(mode=None):
    """Build the single-core Bass program (run SPMD on 8 cores)."""
    mode = MODE if mode is None else mode
    if mode == "f16":
        QD = VD = WTD = OD = F16  # bulk tensors: inputs, exp tiles, out
        PD = F32R                 # on-chip intermediates: qhT/khT/ohT/Wcs
    elif mode == "f32r":
        QD = VD = WTD = OD = PD = F32R
    else:
        QD = VD = WTD = OD = PD = F32

    nc = bacc.Bacc("TRN2", target_bir_lowering=False, debug=False)

    qT = nc.dram_tensor("qT", [D, QB], QD, kind="ExternalInput").ap()
    kT = nc.dram_tensor("kT", [D, S], QD, kind="ExternalInput").ap()
    vT = nc.dram_tensor("vT", [128, S * NDT], VD, kind="ExternalInput").ap()
    Wqkv = nc.dram_tensor(
        "Wqkv", [128, NDT * 3 * DEPTH], QD, kind="ExternalInput").ap()
    Wcs = nc.dram_tensor("Wcs", [DEPTH, D], PD, kind="ExternalInput").ap()
    bq = nc.dram_tensor("bq", [DEPTH, 1], F32, kind="ExternalInput").ap()

    out_u = nc.dram_tensor("out_u", [QB, D], OD, kind="ExternalOutput").ap()
    wT_u = nc.dram_tensor("wT_u", [S, QB], WTD, kind="ExternalOutput").ap()
    sums = nc.dram_tensor("sums", [1, QB], F32, kind="ExternalOutput").ap()

    with ExitStack() as ctx:
        tc = ctx.enter_context(tile.TileContext(nc))
        sb = ctx.enter_context(tc.tile_pool(name="sb", bufs=1))

        # ---------------- constants ----------------
        ident = sb.tile([128, 128], F32, tag="ident")
        make_identity(nc, ident[:])
        eshift_sb = sb.tile([128, 1], F32, tag="eshift")
        nc.gpsimd.memset(eshift_sb[:], -ESHIFT)
        ones_ap = nc.const_aps.tensor(1.0, (128, 1))
        # Dummy exp: forces the Exp LUT load onto ACT while DMAs stream,
        # instead of delaying the first real exp by ~1.3us.
        warm_sb = sb.tile([128, 1], F32, tag="warm")
        nc.scalar.activation(
            warm_sb[:], eshift_sb[:], mybir.ActivationFunctionType.Exp)
        wrm_sb = sb.tile([128, 512], F32, tag="wrm")
        nc.gpsimd.memset(wrm_sb[:], 1.0)

        # ------- input loads, in order of the dependency chains they head ---
        NVC = 4                       # vT arrives in seq quarters
        VQ = S // NVC
        Wqkv_sb = sb.tile([128, NDT, 3 * DEPTH], QD, tag="Wqkv")
        nc.sync.dma_start(
            Wqkv_sb[:], Wqkv.rearrange("p (t d) -> p t d", d=3 * DEPTH))
        qT_r = qT.rearrange("(t p) s -> p t s", p=128)
        qT_sb = sb.tile([128, NDT, QB], QD, tag="qT")
        HT = NDT // 2
        for c in range(2):
            nc.sync.dma_start(
                qT_sb[:, c * HT:(c + 1) * HT, :], qT_r[:, c * HT:(c + 1) * HT, :])
        bq_sb = sb.tile([DEPTH, 1], F32, tag="bq")
        nc.sync.dma_start(bq_sb[:], bq)
        kT_r = kT.rearrange("(t p) s -> p t s", p=128)
        kT_sb = sb.tile([128, NDT, S], QD, tag="kT")
        # vT host-packed quarter-contiguous: [128, (c t s)] so each
        # quarter's DMA moves 3KB runs per partition on both sides
        vT_r = vT.rearrange("p (c t s) -> p c t s", c=NVC, t=NDT)
        vT_sb = sb.tile([128, NVC, NDT, VQ], VD, tag="vT")
        nc.sync.dma_start(kT_sb[:, :, 0:QB], kT_r[:, :, 0:QB])
        nc.sync.dma_start(vT_sb[:, 0], vT_r[:, 0])
        nc.sync.dma_start(kT_sb[:, :, QB:S], kT_r[:, :, QB:S])
        for c in range(1, NVC):
            nc.sync.dma_start(vT_sb[:, c], vT_r[:, c])
        Wcs_sb = sb.tile([DEPTH, D], PD, tag="Wcs")
        nc.sync.dma_start(Wcs_sb[:], Wcs)

        # ---------------- SBUF intermediates ----------------
        qhT_sb = sb.tile([DEPTH, QB], PD, tag="qhT")
        khT_sb = sb.tile([DEPTH, S], PD, tag="khT")
        vhT_sb = sb.tile([DEPTH, S], F32, tag="vhT")
        vha = sb.tile([128, NKT, DEPTH + 1], VD, tag="vha")
        expT = sb.tile([128, NKT, QB], WTD, tag="expT")
        ohT_sb = sb.tile([DEPTH, QB], PD, tag="ohT")
        sums_sb = sb.tile([1, QB], F32, tag="sums")
        o_sb = sb.tile([128, NQT, D], OD, tag="o_sb")

        # ones columns of the augmented-vh tiles (single broadcast, t=0)
        nc.vector.tensor_copy(
            vha[:, :, DEPTH:DEPTH + 1],
            nc.const_aps.tensor(1.0, (128, NKT, 1)))

        # PE warm-up: throwaway fp32 matmuls while the input DMAs stream,
        # so the HAM clock-gate reaches full rate before the first real
        # matmul (the scheduler is not HAM-aware).
        import os as _os2
        NWU = int(_os2.environ.get("MHA_NWU", "3"))
        with tc.tile_pool(name="ps_wu", bufs=1, space="PSUM") as pp_wu:
            wu_ps = pp_wu.tile([128, QB], F32, tag="wu")
            for i in range(NWU):
                nc.tensor.matmul(
                    wu_ps[:], ident[:], wrm_sb[:], start=True, stop=True)

        # ---- whole mid-section on one 8-bank PSUM budget:
        #      acc(4): qh, kh0, sc0..7   pv(2): kh1, vh quarters   tr(2)
        with (
            tc.tile_pool(name="ps_acc", bufs=4, space="PSUM") as pp_acc,
            tc.tile_pool(name="ps_pv", bufs=2, space="PSUM") as pp_pv,
            tc.tile_pool(name="ps_tr", bufs=2, space="PSUM") as pp_tr,
        ):
            # qh (heads the scores chain via qhT copy)
            qh_ps = pp_acc.tile([DEPTH, QB], F32, tag="acc", name="qh")
            for t in range(NDT):
                nc.tensor.matmul(
                    qh_ps[:], Wqkv_sb[:, t, 0:DEPTH], qT_sb[:, t, :],
                    start=(t == 0), stop=(t == NDT - 1),
                )
            nc.scalar.activation(
                qhT_sb[:], qh_ps[:], mybir.ActivationFunctionType.Identity,
                bias=bq_sb[:, 0:1], scale=1.0,
            )
            # kh half 0
            kh0_ps = pp_acc.tile([DEPTH, QB], F32, tag="acc", name="kh0")
            for t in range(NDT):
                nc.tensor.matmul(
                    kh0_ps[:], Wqkv_sb[:, t, DEPTH:2 * DEPTH], kT_sb[:, t, 0:QB],
                    start=(t == 0), stop=(t == NDT - 1),
                )
            nc.scalar.copy(khT_sb[:, 0:QB], kh0_ps[:])

            def scores(k):
                sc_ps = pp_acc.tile([128, QB], F32, tag="acc", name=f"sc{k}")
                nc.tensor.matmul(
                    sc_ps[:], khT_sb[:, k * 128:(k + 1) * 128], qhT_sb[:],
                    start=True, stop=True,
                )
                nc.scalar.activation(
                    expT[:, k, :], sc_ps[:], mybir.ActivationFunctionType.Exp,
                    bias=eshift_sb[:], scale=SCALE,
                )
                if k % (NKT // 2) == NKT // 2 - 1:
                    c = k // (NKT // 2)
                    hk = NKT // 2
                    nc.sync.dma_start(
                        wT_u.rearrange("(k p) s -> p k s", p=128)[
                            :, c * hk:(c + 1) * hk, :],
                        expT[:, c * hk:(c + 1) * hk, :],
                    )

            def vquarter(c):
                sl = slice(c * VQ, (c + 1) * VQ)
                vh_ps = pp_pv.tile([DEPTH, VQ], F32, tag="pv", name=f"vh{c}")
                for t in range(NDT):
                    nc.tensor.matmul(
                        vh_ps[:], Wqkv_sb[:, t, 2 * DEPTH:3 * DEPTH],
                        vT_sb[:, c, t, :],
                        start=(t == 0), stop=(t == NDT - 1),
                    )
                nc.vector.tensor_copy(vhT_sb[:, sl], vh_ps[:])
                # both transposes of the quarter share one PSUM bank; a
                # single DVE copy evacuates them into the vha slab
                KPQ2 = VQ // 128
                k0 = (c * VQ) // 128
                tr_ps = pp_tr.tile([128, KPQ2, DEPTH], F32, tag="tr")
                for kk in range(KPQ2):
                    nc.tensor.transpose(
                        tr_ps[:, kk, :],
                        vhT_sb[:, (k0 + kk) * 128:(k0 + kk + 1) * 128],
                        ident[0:DEPTH, 0:DEPTH],
                    )
                nc.vector.tensor_copy(
                    vha[:, k0:k0 + KPQ2, 0:DEPTH], tr_ps[:])

            for k in range(NKT // 2):
                scores(k)
            vquarter(0)
            # kh half 1
            kh1_ps = pp_pv.tile([DEPTH, QB], F32, tag="pv", name="kh1")
            for t in range(NDT):
                nc.tensor.matmul(
                    kh1_ps[:], Wqkv_sb[:, t, DEPTH:2 * DEPTH], kT_sb[:, t, QB:S],
                    start=(t == 0), stop=(t == NDT - 1),
                )
            nc.vector.tensor_copy(khT_sb[:, QB:S], kh1_ps[:])
            for k in range(NKT // 2, NKT):
                scores(k)
            for c in range(1, NVC):
                vquarter(c)

        # ---------------- AV + sums, output projection ----------------
        with tc.tile_pool(name="ps_o", bufs=1, space="PSUM") as pp_o:
            oh_ps = pp_o.tile([DEPTH + 1, QB], F32, tag="oh")
            for k in range(NKT):
                nc.tensor.matmul(
                    oh_ps[:], vha[:, k, :], expT[:, k, :],
                    start=(k == 0), stop=(k == NKT - 1),
                )
            # ohT copy split across ACT and DVE halves
            nc.scalar.copy(ohT_sb[:, 0:QB // 2], oh_ps[0:DEPTH, 0:QB // 2])
            nc.vector.tensor_copy(
                ohT_sb[:, QB // 2:QB], oh_ps[0:DEPTH, QB // 2:QB])

            with tc.tile_pool(name="ps_out", bufs=3, space="PSUM") as pp_out:
                out_r = out_u.rearrange("(t p) d -> p t d", p=128)
                for qt in range(NQT):
                    o_ps = pp_out.tile([128, D], F32, tag="o")
                    qsl = slice(qt * 128, (qt + 1) * 128)
                    nc.tensor.matmul(
                        o_ps[:, 0:512], ohT_sb[:, qsl], Wcs_sb[:, 0:512],
                        start=True, stop=True,
                    )
                    nc.tensor.matmul(
                        o_ps[:, 512:D], ohT_sb[:, qsl], Wcs_sb[:, 512:D],
                        start=True, stop=True,
                    )
                    if qt % 2 == 0:
                        nc.vector.tensor_copy(o_sb[:, qt, :], o_ps[:])
                    else:
                        nc.scalar.copy(o_sb[:, qt, :], o_ps[:])
                    if qt % 2 == 1:
                        c = qt // 2
                        nc.sync.dma_start(
                            out_r[:, 2 * c:2 * c + 2, :], o_sb[:, 2 * c:2 * c + 2, :])
                # sums copy last - it is off the critical path (host-only)
                nc.vector.tensor_copy(sums_sb[:], oh_ps[DEPTH:DEPTH + 1, :])
                nc.sync.dma_start(sums, sums_sb[:])

    nc.compile()
    return nc


_PROGRAM = {}


def _get_program(mode=None):
    key = MODE if mode is None else mode
    if key not in _PROGRAM:
        _PROGRAM[key] = build_bass(key)
    return _PROGRAM[key]


def make_in_maps(q, k, v, Wq, bq, Wk, Wv, Wcs, mode=None):
    """Host-side sharding: per-core input dicts (transposed slabs)."""
    mode = MODE if mode is None else mode
    adt = np.float16 if mode == "f16" else np.float32
    kT = [np.ascontiguousarray(k[b].T, dtype=adt) for b in range(B)]
    vT = [
        np.ascontiguousarray(
            np.asarray(v[b].T, dtype=adt).reshape(6, 128, 4, 256)
            .transpose(1, 2, 0, 3).reshape(128, -1))
        for b in range(B)
    ]
    Wqkv = np.concatenate([Wq, Wk, Wv], axis=1).astype(adt)
    # pre-swizzle to the SBUF layout: [128 partitions, (t d)] so the DMA
    # reads fully contiguous 2.3KB runs (the transpose-gather AP would
    # produce 384B runs, below the 512B DMA line-rate threshold)
    Wqkv = np.ascontiguousarray(
        Wqkv.reshape(NDT, 128, 3 * DEPTH).transpose(1, 0, 2).reshape(
            128, NDT * 3 * DEPTH))
    Wcs = np.ascontiguousarray(Wcs, dtype=np.float32)
    bq = np.ascontiguousarray(np.asarray(bq, dtype=np.float32).reshape(DEPTH, 1))
    in_maps = []
    for c in range(NCORES):
        b, half = divmod(c, 2)
        qT_c = np.ascontiguousarray(q[b, half * QB:(half + 1) * QB, :].T, dtype=adt)
        in_maps.append({
            "qT": qT_c, "kT": kT[b], "vT": vT[b],
            "Wqkv": Wqkv, "Wcs": Wcs, "bq": bq,
        })
    return in_maps


def assemble(results, Wcs, bv, bc):
    """Host-side gather + normalization. results: list of 8 per-core dicts."""
    out = np.empty((B, S, D), np.float32)
    w = np.empty((B, S, S), np.float32)
    out_bias = (np.asarray(bv, np.float32) @ Wcs + np.asarray(bc, np.float32)).astype(
        np.float32
    )
    for c in range(NCORES):
        b, half = divmod(c, 2)
        r = results[c]
        recip = (np.float32(1.0) / r["sums"][0]).astype(np.float32)[:, None]
        sl = slice(half * QB, (half + 1) * QB)
        out[b, sl, :] = r["out_u"].astype(np.float32) * recip + out_bias
        w[b, sl, :] = r["wT_u"].astype(np.float32).T * recip
    attn = np.broadcast_to(w[:, None, :, :], (B, H, S, S))
    return out, attn


def kernel(q, k, v, Wq, bq, Wk, bk, Wv, bv, Wc, bc):
    from concourse.bass_utils import run_bass_kernel_spmd

    q = np.asarray(q, np.float32)
    k = np.asarray(k, np.float32)
    v = np.asarray(v, np.float32)
    Wc = np.asarray(Wc, np.float32)
    Wcs = np.ascontiguousarray(Wc.reshape(H, DEPTH, D).sum(axis=0, dtype=np.float32))

    nc = _get_program()
    in_maps = make_in_maps(q, k, v, Wq, bq, Wk, Wv, Wcs)
    res = run_bass_kernel_spmd(nc, in_maps, core_ids=list(range(NCORES)))
    return assemble(res.results, Wcs, bv, bc)
